# revision 36
# baseline (speedup 1.0000x reference)
"""Trainium2 Bass kernel for CausalSelfAttention (GQA + per-head RMS norm + RoPE).

Sharding: 8 cores = batch(2) x kv-head-group(4). Each core computes, for its
(b, g): qkv projection (its 4 rep q heads + 1 kv head), per-head RMS norm,
RoPE, causal attention, and a partial output projection (its 512 rows of
w_proj). Host sums the 4 partial projections per batch element.

v3 design notes (vs the v2 f16 baseline at 299us):
  - Host pre-normalizes x (token rms_norm commutes out of q/k entirely; v
    needs x-hat directly), so the 8MB/core untransposed-x input, its ACT
    square pass, and the v rescale all disappear.  Input DMA drops 23->15MB,
    fixing the DMA-saturated 21us startup stall.
  - wqkv streams as 16 per-kt slices on two queues in consumption order so
    tile-0's accumulation never waits on the bulk transfer.
  - Chunk-0 attention is woven into the tail of phase 1 (its deps are token
    tiles 0-3 only), removing the phase-boundary PE gap and the HAM
    re-throttle it caused.
  - Attention emission is split into S-steps (score MMs + exp + mask + den)
    and V-steps (attnV MMs), software-pipelined across the two heads of a
    pair so each V sits >=2.5us of PE work after its own exp.
  - Softmax denominator: partition-sum and broadcast fused into ONE matmul
    with an all-ones [128,128] stationary operand (out rows all equal the
    column sum), dropping the two-matmul ds/broadcast chain.
  - den accumulation: bulk full-width adds on GpSimd (otherwise idle in
    phase 2), ragged/diagonal adds + folds on DVE; causal masks on DVE
    (cheap 2x f16) instead of GpSimd; all y PSUM->SBUF copies on DVE so
    ACT does (almost) nothing but exp.
"""

import functools
import os

import numpy as np

from concourse import bacc, bass, mybir
from concourse import tile
from concourse.bass_utils import run_bass_kernel_spmd

# The activation-table pass binds exp to `exp_and_others` even when
# `natural_log_exp_and_others` (which also has ln + square) covers every
# function this kernel uses, causing a table reload between each ln and exp.
# Restrict exp/ln to the combined set (set order, hence set ids, preserved)
# so the whole kernel runs on one table load.
_orig_get_activation_tables = bacc.get_activation_tables


@functools.cache
def _patched_get_activation_tables(arch):
    t = dict(_orig_get_activation_tables(arch))
    keep = "natural_log_exp_and_others"
    if keep in t:
        AFT = mybir.ActivationFunctionType
        for k in t:
            if k != keep:
                t[k] = t[k] - {AFT.Exp, AFT.Ln}
    return t


bacc.get_activation_tables = _patched_get_activation_tables

# Problem shape (hardcoded per contract)
B, T, C = 2, 2048, 2048
N_HEADS, N_KV = 16, 4
HD = C // N_HEADS            # 128
REP = N_HEADS // N_KV        # 4
KV_DIM = N_KV * HD           # 512
P = 128
TT = T // P                  # 16 token tiles
KT = C // P                  # 16 contraction tiles
JQ = REP * HD                # 512 local q cols
JTOT = JQ + 2 * HD           # 768 local qkv cols
TCW = 512                    # attention t-chunk width
NTC = T // TCW               # 4
EPS = 1.1920929e-07
SCALE = 1.0 / float(np.sqrt(HD))
EXPBIAS = -9.0               # et = exp(s*SCALE - 9) stays in f16 range

F32 = mybir.dt.float32
F16 = mybir.dt.float16
AF = mybir.ActivationFunctionType
AX = mybir.AxisListType


def _emit(nc):
    # xt[tt, p, kt*128 + j] = xhat[tt*128 + j, kt*128 + p]  (pre-tiled lhsT,
    # host-normalized: xhat = x * rstd(token))
    xt_d = nc.dram_tensor("xt", [TT, P, C], F16, kind="ExternalInput")
    # wqkv[p, kt, j] = w_qkv[kt*128 + p, j]; j = [q 512 | k 128 | v 128]
    wqkv_d = nc.dram_tensor("wqkv", [P, KT, JTOT], F16, kind="ExternalInput")
    # wproj[p, h, c] = w_proj[h*128 + p, c]
    wproj_d = nc.dram_tensor("wproj", [P, REP, C], F16, kind="ExternalInput")
    gain_d = nc.dram_tensor("gain", [1, REP], F32, kind="ExternalInput")
    # rope tables pre-expanded to all 5 lanes (4 q heads + k):
    # cos5[tt, p, j*HD+d] = cos(tt*128+p, d)
    cos_d = nc.dram_tensor("costab", [TT, P, 5 * HD], F16, kind="ExternalInput")
    sin_d = nc.dram_tensor("sintab", [TT, P, 5 * HD], F16, kind="ExternalInput")  # [:, :, :64] = -sin
    bmask_d = nc.dram_tensor("bmask", [P, P], F16, kind="ExternalInput")  # 0/1
    id_d = nc.dram_tensor("ident", [P, P], F16, kind="ExternalInput")
    y_d = nc.dram_tensor("y", [T, C], F16, kind="ExternalOutput")

    with tile.TileContext(nc) as tc:
        with tc.tile_pool(name="persist", bufs=1) as pp, \
             tc.tile_pool(name="psum", bufs=1, space="PSUM") as psp, \
             nc.allow_low_precision(reason="f16 kernel by design"):
            # Long-lived f16 activations
            qTall = pp.tile([P, REP, T], F16, name="qTall", tag="qTall")
            kTt = pp.tile([P, T], F16, name="kTt", tag="kTt")
            vN = pp.tile([P, TT, HD], F16, name="vN", tag="vN")
            bmask_sb = pp.tile([P, P], F16, name="bmask_sb", tag="bmask")
            nc.gpsimd.dma_start(out=bmask_sb, in_=bmask_d.ap())
            negb = pp.tile([P, 1], F32, name="negb", tag="negb")
            nc.vector.memset(negb, EXPBIAS)
            ones128 = pp.tile([P, P], F16, name="ones128", tag="ones128")
            nc.vector.memset(ones128, 1.0)
            wproj_sb = pp.tile([P, REP, C], F16, name="wproj_sb", tag="wproj")

            # ---------------- Phase 1: qkv + norms + rope + transposes -------
            with tc.tile_pool(name="ph1", bufs=1) as p1:
                wqkv_sb = p1.tile([P, KT, JTOT], F16, name="wqkv_sb", tag="wqkv")
                id_sb = p1.tile([P, P], F16, name="id_sb", tag="ident")
                nc.gpsimd.dma_start(out=id_sb, in_=id_d.ap())
                eps_t = p1.tile([P, 1], F32, name="eps_t", tag="eps")
                nc.vector.memset(eps_t, EPS)

                # HAM warm-up: the first ~13us are DMA/preamble-bound with
                # PE idle, so the clock gate would hold the PE at 1.2GHz for
                # the first ~3.4us of real work.  Chew cheap matmuls on the
                # memset ones tile (no DMA dependency -- starts the moment
                # the framework preamble ends) to flip HAM to 8/8 and keep
                # it there until tile 0's inputs land.
                warm_z = p1.tile([P, P], F16, name="warm_z", tag="warm_z")
                nc.vector.memset(warm_z, 0.0)
                warm_sb = p1.tile([P, 1], F32, name="warm_sb", tag="warm_sb")
                for g in range(4):
                    warm_ps = psp.tile([P, P], F32, name=f"warm_ps{g}",
                                       tag="small", bufs=2)
                    for i in range(22):
                        nc.tensor.matmul(warm_ps, warm_z, warm_z,
                                         start=(i == 0), stop=(i == 21))
                    nc.vector.tensor_copy(warm_sb, warm_ps[:, 0:1])
                # preload the ln/exp table set (the only set this kernel
                # uses) while ACT is otherwise idle
                nc.scalar.activation(warm_sb, eps_t, AF.Ln)

                # broadcast gain [1,4] -> [128,4] via 0-stride DMA
                # replication (DMA emitted after the critical startup DMAs)
                gainb = p1.tile([P, REP], F32, name="gainb", tag="gainb")


                # chunk-0 attention woven into the phase-1 tail: S-steps and
                # V-steps land on different tiles so each exp has a full
                # tile (~6us) of latency cover; attnV drains per-block to an
                # SBUF accumulator so no long-lived PSUM tile ever blocks
                # the per-tile q_ps/kv_ps/tq rotations.
                def chunk0_steps():
                    for h in range(REP):
                        osb = pp.tile([P, TCW], F32, name=f"c0osb_{h}",
                                      tag="c0osb", bufs=2)
                        sA, vA, tail = attend_plan(0, h, None, c0_osb=osb)
                        yield sA[0]
                        yield sA[1]
                        yield vA[0]
                        yield vA[1]
                        yield tail

                c0_iter = None

                # Token tiles in an order that retires the late-chunk tiles
                # (12-15) early: the phase boundary then depends only on
                # tiles that finished long ago, and chunk 0/1 deps (tiles
                # 0-7) are ready the moment phase 2 starts.
                proc_order = [0, 1, 2, 3, 12, 13, 14, 15,
                              4, 5, 6, 7, 8, 9, 10, 11]

                # Startup DMA priority: tile-0's xt on the otherwise-idle
                # sync queue; ALL wqkv kt-slices next (tile 0 consumes them
                # in order); the next two xt tiles land behind wqkv on the
                # same queues so they cannot steal its bandwidth.
                xT_tiles = {}

                def xT_dma(oi, qeng):
                    tt = proc_order[oi]
                    xt = p1.tile([P, C], F16, name=f"xT_{tt}", tag="xT", bufs=3)
                    qeng.dma_start(out=xt, in_=xt_d.ap()[tt])
                    xT_tiles[oi] = xt

                xT_dma(0, nc.sync)
                for ks in range(KT // 2):
                    qeng = (nc.scalar, nc.gpsimd)[ks % 2]
                    qeng.dma_start(out=wqkv_sb[:, 2 * ks:2 * ks + 2, :],
                                   in_=wqkv_d.ap()[:, 2 * ks:2 * ks + 2, :])
                xT_dma(1, nc.scalar)
                xT_dma(2, nc.gpsimd)
                nc.scalar.dma_start(out=gainb,
                                    in_=gain_d.ap()[0].partition_broadcast(P))

                tpq = []  # software-pipelined q/k transposes (depth 2)
                H2 = HD // 2

                def emit_qk_transposes(qkf_t, ptt):
                    tq = psp.tile([P, JQ], F16, name=f"tq_{ptt}", tag="mm", bufs=2)
                    for h in range(REP):
                        nc.tensor.transpose(tq[:, h * P:(h + 1) * P],
                                            qkf_t[:, h * P:(h + 1) * P], id_sb)
                    # ONE strided PSUM->SBUF copy for all 4 heads on ACT
                    # ('copy' is in every table set)
                    nc.scalar.copy(qTall[:, :, ptt * P:(ptt + 1) * P],
                                   tq.rearrange("p (h c) -> p h c", h=REP))
                    tk = psp.tile([P, HD], F16, name=f"tk_{ptt}", tag="small", bufs=2)
                    nc.tensor.transpose(tk, qkf_t[:, JQ:JQ + HD], id_sb)
                    nc.scalar.copy(kTt[:, ptt * P:(ptt + 1) * P], tk)

                ao_tiles = {}

                def attend_plan(tci, h, o_ps, c0_osb=None, o_blocks=None):
                    """Return (s_steps, v_steps, tail) closures for (tci, h).

                    s_steps[k]: score MMs + exp (+ causal mask) + den update
                    v_steps[k]: the two attnV MMs consuming et[k]
                    tail: den fold + fused partition-sum/broadcast + rescale

                    The last 4 s-tiles form the diagonal block: their score /
                    attnV matmuls are column-sliced to the causal range
                    (widths 512/384/256/128) and only the leading [128,128]
                    triangle of each strip needs masking.

                    c0_osb: chunk-0 weave mode -- attnV uses transient psum
                    blocks drained into this SBUF f32 accumulator so the
                    phase-1 psum rotations never block on a long-lived
                    accumulator."""
                    nst = 4 * (tci + 1)
                    nfull = nst - 4
                    denf = pp.tile([P, 2 * TCW], F16, name=f"dnf_{tci}_{h}",
                                   tag="denf", bufs=4)
                    qTc = qTall[:, h, tci * TCW:(tci + 1) * TCW]
                    s_steps, v_steps = [], []
                    for sw in range(nfull // 2):
                        st0, st1 = 2 * sw, 2 * sw + 1
                        sc = psp.tile([P, 2 * TCW], F32, name=f"sc_{tci}_{h}_{sw}",
                                      tag="mm", bufs=2)
                        et = pp.tile([P, 2 * TCW], F16, name=f"et_{tci}_{h}_{sw}",
                                     tag="et", bufs=6)

                        def s_fn(sc=sc, et=et, st0=st0, st1=st1, sw=sw):
                            nc.tensor.matmul(sc[:, 0:TCW],
                                             kTt[:, st0 * P:(st0 + 1) * P],
                                             qTc, start=True, stop=True)
                            nc.tensor.matmul(sc[:, TCW:],
                                             kTt[:, st1 * P:(st1 + 1) * P],
                                             qTc, start=True, stop=True)
                            nc.scalar.activation(et, sc, AF.Exp, scale=SCALE,
                                                 bias=negb)
                            if sw == 0:
                                nc.vector.tensor_copy(denf, et)
                            else:
                                nc.vector.tensor_add(denf, denf, et)

                        def v_fn(et=et, st0=st0, st1=st1, sw=sw):
                            nc.tensor.matmul(o_ps, vN[:, st0, :], et[:, 0:TCW],
                                             start=(sw == 0), stop=False)
                            nc.tensor.matmul(o_ps, vN[:, st1, :], et[:, TCW:],
                                             start=False, stop=False)

                        s_steps.append(s_fn)
                        v_steps.append(v_fn)
                    first = (nfull == 0)
                    for pi, (v0, v1) in enumerate(((0, 1), (2, 3))):
                        st0, st1 = 4 * tci + v0, 4 * tci + v1
                        t0, t1 = v0 * P, v1 * P
                        w0, w1 = TCW - t0, TCW - t1
                        sc = psp.tile([P, 2 * TCW], F32, name=f"scd_{tci}_{h}_{pi}",
                                      tag="mm", bufs=2)
                        et = pp.tile([P, 2 * TCW], F16, name=f"etd_{tci}_{h}_{pi}",
                                     tag="et", bufs=6)

                        def s_fn(sc=sc, et=et, st0=st0, st1=st1, pi=pi,
                                 t0=t0, t1=t1, w0=w0, w1=w1):
                            nc.tensor.matmul(sc[:, 0:w0],
                                             kTt[:, st0 * P:(st0 + 1) * P],
                                             qTc[:, t0:TCW], start=True, stop=True)
                            nc.tensor.matmul(sc[:, w0:w0 + w1],
                                             kTt[:, st1 * P:(st1 + 1) * P],
                                             qTc[:, t1:TCW], start=True, stop=True)
                            nc.scalar.activation(et[:, 0:w0 + w1], sc[:, 0:w0 + w1],
                                                 AF.Exp, scale=SCALE, bias=negb)
                            # ragged triangle masks: first 128 cols of each
                            # strip, on GpSimd (otherwise idle in phase 2)
                            nc.gpsimd.tensor_mul(et[:, 0:P], et[:, 0:P], bmask_sb)
                            nc.gpsimd.tensor_mul(et[:, w0:w0 + P],
                                                 et[:, w0:w0 + P], bmask_sb)
                            if first and pi == 0:
                                nc.vector.tensor_copy(denf[:, 0:TCW], et[:, 0:TCW])
                                nc.vector.memset(denf[:, TCW:TCW + t1], 0.0)
                                nc.vector.tensor_copy(denf[:, TCW + t1:2 * TCW],
                                                      et[:, w0:w0 + w1])
                            else:
                                nc.vector.tensor_add(denf[:, t0:TCW],
                                                     denf[:, t0:TCW], et[:, 0:w0])
                                nc.vector.tensor_add(denf[:, TCW + t1:2 * TCW],
                                                     denf[:, TCW + t1:2 * TCW],
                                                     et[:, w0:w0 + w1])

                        if c0_osb is None:
                            def v_fn(et=et, st0=st0, st1=st1, pi=pi,
                                     t0=t0, t1=t1, w0=w0, w1=w1):
                                nc.tensor.matmul(o_ps[:, t0:TCW], vN[:, st0, :],
                                                 et[:, 0:w0],
                                                 start=(first and pi == 0),
                                                 stop=False,
                                                 skip_group_check=True)
                                nc.tensor.matmul(o_ps[:, t1:TCW], vN[:, st1, :],
                                                 et[:, w0:w0 + w1],
                                                 start=False, stop=(pi == 1),
                                                 skip_group_check=True)
                        else:
                            def v_fn(et=et, st0=st0, st1=st1, pi=pi,
                                     t0=t0, t1=t1, w0=w0, w1=w1):
                                o_blk = psp.tile([P, TCW], F32,
                                                 name=f"c0o_{h}_{pi}",
                                                 tag="acc", bufs=2)
                                nc.tensor.matmul(o_blk[:, t0:TCW], vN[:, st0, :],
                                                 et[:, 0:w0],
                                                 start=True, stop=False,
                                                 skip_group_check=True)
                                nc.tensor.matmul(o_blk[:, t1:TCW], vN[:, st1, :],
                                                 et[:, w0:w0 + w1],
                                                 start=False, stop=True,
                                                 skip_group_check=True)
                                if pi == 0:
                                    nc.vector.tensor_copy(c0_osb, o_blk)
                                else:
                                    nc.vector.tensor_add(c0_osb[:, t0:TCW],
                                                         c0_osb[:, t0:TCW],
                                                         o_blk[:, t0:TCW])

                        s_steps.append(s_fn)
                        v_steps.append(v_fn)

                    def tail():
                        den_r = pp.tile([P, TCW], F16, name=f"dnr_{tci}_{h}",
                                        tag="denr", bufs=4)
                        nc.vector.tensor_add(den_r, denf[:, 0:TCW], denf[:, TCW:])
                        # all-ones stationary: every out row = column sum(den_r)
                        rb_ps = psp.tile([P, TCW], F32, name=f"rb_{tci}_{h}",
                                         tag="small", bufs=2)
                        nc.tensor.matmul(rb_ps, ones128, den_r, start=True,
                                         stop=True)
                        rb = pp.tile([P, TCW], F32, name=f"rbs_{tci}_{h}",
                                     tag="rb", bufs=4)
                        nc.vector.reciprocal_approx_fast(rb, rb_ps)
                        aot = pp.tile([P, TCW], F16, name=f"ao_{tci}_{h}",
                                      tag="ao", bufs=8)
                        osrc = o_ps if c0_osb is None else c0_osb
                        nc.vector.tensor_mul(aot, osrc, rb)
                        ao_tiles[(tci, h)] = aot

                    return s_steps, v_steps, tail

                def pair_group_steps(tci, hp):
                    """Emission schedule for heads (hp, hp+1) of chunk tci.

                    S/V stages are split and cross-staggered so every V sits
                    ~2.5us of PE work after its own S (exp+mask latency)."""
                    o0 = psp.tile([P, TCW], F32, name=f"ops_{tci}_{hp}",
                                  tag="acc", bufs=2)
                    o1 = psp.tile([P, TCW], F32, name=f"ops_{tci}_{hp + 1}",
                                  tag="acc", bufs=2)
                    sA, vA, tA = attend_plan(tci, hp, o0)
                    sB, vB, tB = attend_plan(tci, hp + 1, o1)
                    n = len(sA)
                    sA[0]()
                    sB[0]()
                    yield
                    for k in range(n):
                        vA[k]()
                        if k + 1 < n:
                            sA[k + 1]()
                        yield
                        vB[k]()
                        if k + 1 < n:
                            sB[k + 1]()
                        yield
                    tA()
                    yield
                    tB()
                    yield

                def proj_steps(tci):
                    """Projection for chunk tci as small emit-steps (4 MMs each)."""
                    for ttl in range(4):
                        yt = pp.tile([P, C], F16, name=f"y_{tci}_{ttl}", tag="y", bufs=2)
                        for ncs in range(4):
                            def step(tci=tci, ttl=ttl, ncs=ncs, yt=yt):
                                y_ps = psp.tile([P, 512], F32,
                                                name=f"yps_{tci}_{ttl}_{ncs}",
                                                tag="small", bufs=2)
                                for h in range(REP):
                                    nc.tensor.matmul(
                                        y_ps,
                                        ao_tiles[(tci, h)][:, ttl * P:(ttl + 1) * P],
                                        wproj_sb[:, h, ncs * 512:(ncs + 1) * 512],
                                        start=(h == 0), stop=(h == REP - 1))
                                # alternate DVE/ACT for the PSUM->SBUF y copies
                                if ncs % 2 == 0:
                                    nc.vector.tensor_copy(
                                        yt[:, ncs * 512:(ncs + 1) * 512], y_ps)
                                else:
                                    nc.scalar.copy(
                                        yt[:, ncs * 512:(ncs + 1) * 512], y_ps)
                                if tci == NTC - 1:
                                    # last chunk: DMA per column slice so the
                                    # trailing drain after the final matmul
                                    # is short
                                    row = (tci * 4 + ttl) * P
                                    qeng = (nc.sync, nc.gpsimd, nc.sync,
                                            nc.gpsimd)[ncs]
                                    qeng.dma_start(
                                        out=y_d.ap()[row:row + P,
                                                     ncs * 512:(ncs + 1) * 512],
                                        in_=yt[:, ncs * 512:(ncs + 1) * 512])
                            yield step
                        if tci != NTC - 1:
                            def dma_step(tci=tci, ttl=ttl, yt=yt):
                                row = (tci * 4 + ttl) * P
                                qeng = (nc.sync, nc.gpsimd, nc.sync, nc.sync)[ttl]
                                qeng.dma_start(out=y_d.ap()[row:row + P, :], in_=yt)
                            yield dma_step

                NQK = REP + 1     # merged q-heads + k lane count

                for oi in range(TT):
                    tt = proc_order[oi]
                    if oi + 3 < TT:
                        # bufs=3 rotation throttles this DMA until tile oi's
                        # buffer frees, so prefetches never starve wqkv
                        xT_dma(oi + 3, nc.sync)
                    xT_t = xT_tiles.pop(oi)
                    cos_t = p1.tile([P, NQK * HD], F16, name=f"cos_{tt}",
                                    tag="cos", bufs=2)
                    sin_t = p1.tile([P, NQK * HD], F16, name=f"sin_{tt}",
                                    tag="sin", bufs=2)
                    ceng = nc.gpsimd if oi < 2 else nc.sync
                    ceng.dma_start(out=cos_t, in_=cos_d.ap()[tt])
                    ceng.dma_start(out=sin_t, in_=sin_d.ap()[tt])
                    if 8 <= oi <= 11:  # proj weights, one head per tile so a
                        # single large transfer never backs up the DMA engines
                        h = oi - 8
                        qeng = (nc.scalar, nc.gpsimd)[h % 2]
                        qeng.dma_start(out=wproj_sb[:, h:h + 1, :],
                                       in_=wproj_d.ap()[:, h:h + 1, :])

                    # qkv matmuls: q_ps [P, 512], kv_ps [P, 256]
                    q_ps = psp.tile([P, JQ], F32, name=f"qps_{tt}", tag="acc", bufs=2)
                    kv_ps = psp.tile([P, 2 * HD], F32, name=f"kvps_{tt}", tag="small", bufs=2)
                    for kt in range(KT):
                        lb = xT_t[:, kt * P:(kt + 1) * P]
                        nc.tensor.matmul(q_ps, lb, wqkv_sb[:, kt, 0:JQ],
                                         start=(kt == 0), stop=(kt == KT - 1))
                        nc.tensor.matmul(kv_ps, lb, wqkv_sb[:, kt, JQ:JTOT],
                                         start=(kt == 0), stop=(kt == KT - 1))

                    # 2-tiles-back q/k transposes overlap this tile's matmuls
                    if len(tpq) >= 2:
                        emit_qk_transposes(*tpq.pop(0))

                    # ---- q: per-head rms norm (x gain) ----
                    qsq = p1.tile([P, JQ], F16, name=f"qsq_{tt}", tag="qsq", bufs=2)
                    nc.scalar.activation(qsq, q_ps, AF.Square)
                    ssq4 = p1.tile([P, REP], F32, name=f"ssq4_{tt}", tag="ssq4", bufs=2)
                    nc.vector.reduce_sum(ssq4, qsq.rearrange("p (h d) -> p h d", h=REP),
                                         axis=AX.X)
                    # rstd = exp(-0.5*ln(ms+eps)): keeps ACT on the single
                    # ln/exp table set (Sqrt would force a table reload)
                    lq = p1.tile([P, REP], F32, name=f"lq_{tt}", tag="lq", bufs=2)
                    nc.scalar.activation(lq, ssq4, AF.Ln, scale=1.0 / HD, bias=eps_t)
                    rstdq = p1.tile([P, REP], F32, name=f"rstdq_{tt}", tag="rstdq", bufs=2)
                    nc.scalar.activation(rstdq, lq, AF.Exp, scale=-0.5)
                    rstdqg = p1.tile([P, REP], F32, name=f"rstdqg_{tt}", tag="rstdqg", bufs=2)
                    nc.vector.tensor_mul(rstdqg, rstdq, gainb)

                    # ---- k: rms norm (ACT sums) ----
                    scrk = p1.tile([P, HD], F16, name=f"scrk_{tt}", tag="scrk", bufs=2)
                    ssk = p1.tile([P, 1], F32, name=f"ssk_{tt}", tag="ssk", bufs=2)
                    nc.scalar.activation(scrk, kv_ps[:, 0:HD], AF.Square,
                                         accum_out=ssk)
                    lk = p1.tile([P, 1], F32, name=f"lk_{tt}", tag="lk", bufs=2)
                    nc.scalar.activation(lk, ssk, AF.Ln, scale=1.0 / HD, bias=eps_t)
                    rstdk = p1.tile([P, 1], F32, name=f"rstdk_{tt}", tag="rstdk", bufs=2)
                    nc.scalar.activation(rstdk, lk, AF.Exp, scale=-0.5)

                    # ---- merged q+k normalize into one [P, 5*HD] tile, then
                    # one rope pass over all 5 lanes (DVE f-mul, GpSimd
                    # B-muls + add) ----
                    qkn_t = p1.tile([P, NQK * HD], F16, name=f"qkn_{tt}", tag="qkn", bufs=2)
                    # all 4 q heads scaled in one op (rstdqg broadcast along d)
                    nc.vector.tensor_mul(
                        qkn_t[:, 0:JQ].rearrange("p (h d) -> p h d", h=REP),
                        q_ps.rearrange("p (h d) -> p h d", h=REP),
                        rstdqg[:, :, None].broadcast_to([P, REP, HD]))
                    nc.vector.tensor_scalar_mul(qkn_t[:, JQ:JQ + HD],
                                                kv_ps[:, 0:HD], rstdk)
                    qkn3 = qkn_t.rearrange("p (h d) -> p h d", h=NQK)
                    qkf_t = p1.tile([P, NQK * HD], F16, name=f"qkf_{tt}", tag="qkf", bufs=3)
                    qkf3 = qkf_t.rearrange("p (h d) -> p h d", h=NQK)
                    qkB_t = p1.tile([P, NQK * HD], F16, name=f"qkB_{tt}", tag="qkB", bufs=2)
                    qkB3 = qkB_t.rearrange("p (h d) -> p h d", h=NQK)
                    sin3 = sin_t.rearrange("p (h d) -> p h d", h=NQK)
                    # full rope on DVE at 2x f16 rate (expanded tables keep
                    # every operand contiguous)
                    nc.vector.tensor_mul(qkf_t, qkn_t, cos_t)
                    nc.vector.tensor_mul(qkB3[:, :, 0:H2], qkn3[:, :, H2:HD],
                                         sin3[:, :, 0:H2])
                    nc.vector.tensor_mul(qkB3[:, :, H2:HD], qkn3[:, :, 0:H2],
                                         sin3[:, :, H2:HD])
                    nc.vector.tensor_add(qkf_t, qkf_t, qkB_t)

                    # ---- v: already token-normalized on host; just cast ----
                    nc.vector.tensor_copy(vN[:, tt, :], kv_ps[:, HD:2 * HD])

                    tpq.append((qkf_t, tt))

                    if oi == 5:
                        c0_iter = chunk0_steps()
                    if c0_iter is not None:
                        for _ in range(2):
                            st = next(c0_iter, None)
                            if st is not None:
                                st()

                # the last two tiles' transposes are only needed by chunk 2;
                # weave them into early phase 2 so the first score matmuls
                # don't rotate PSUM slots behind them
                deferred_tq = [
                    (lambda a=args: emit_qk_transposes(*a)) for args in tpq]
                tpq.clear()
                if c0_iter is not None:
                    for st in c0_iter:
                        st()

            # ---------------- Phase 2+3: attention + projection --------------
            if True:

                class Weaver:
                    """Spread a step list evenly over a known yield count so
                    late pair-tails still have PE filler work."""

                    def __init__(self, steps, yields):
                        self.steps = list(steps)
                        self.y = max(yields, 1)
                        self.i = 0
                        self.t = 0

                    def tick(self):
                        self.t += 1
                        want = min(len(self.steps),
                                   (len(self.steps) * self.t) // self.y)
                        while self.i < want:
                            self.steps[self.i]()
                            self.i += 1

                    def flush(self):
                        while self.i < len(self.steps):
                            self.steps[self.i]()
                            self.i += 1

                def chunk_yields(tci):
                    return 8 * tci + 14

                wv = Weaver(deferred_tq + list(proj_steps(0)),
                            chunk_yields(1))
                for tci in range(1, NTC):
                    for hp in (0, 2):
                        for _ in pair_group_steps(tci, hp):
                            wv.tick()
                    wv.flush()
                    wv = Weaver(proj_steps(tci),
                                chunk_yields(tci + 1) if tci + 1 < NTC else 1)
                wv.flush()

    return nc


_NC_CACHE = {}
LAST_RESULT = None


def _get_nc():
    if "v3" not in _NC_CACHE:
        nc = bacc.Bacc("TRN2", target_bir_lowering=False, debug=False)
        _emit(nc)
        nc.compile()
        _NC_CACHE["v3"] = nc
    return _NC_CACHE["v3"]


def _host_tables():
    inv_freq = 1.0 / (10000.0 ** (np.arange(0, HD, 2, dtype=np.float64) / HD))
    t = np.arange(T, dtype=np.float64)
    freqs = np.outer(t, inv_freq)                      # [T, 64]
    emb = np.concatenate([freqs, freqs], axis=-1)      # [T, 128]
    cos = np.cos(emb).astype(np.float16)
    sin = np.sin(emb).astype(np.float32)
    sin_signed = sin.copy()
    sin_signed[:, :HD // 2] *= -1.0                    # first half gets -sin
    sin_signed = sin_signed.astype(np.float16)
    # expand to the 5 merged rope lanes (4 q heads + k), tiled by token tile
    cos5 = np.tile(cos, (1, 5)).reshape(TT, P, 5 * HD)
    sin5 = np.tile(sin_signed, (1, 5)).reshape(TT, P, 5 * HD)
    # binary causal triangle for the leading 128 cols of each diagonal strip
    s = np.arange(P)[:, None]
    tcol = np.arange(P)[None, :]
    bmask = (s <= tcol).astype(np.float16)
    ident = np.eye(P, dtype=np.float16)
    return np.ascontiguousarray(cos5), np.ascontiguousarray(sin5), bmask, ident


def kernel(x, w_qkv, w_proj, q_gain):
    global LAST_RESULT
    x = np.asarray(x, dtype=np.float32)
    w_qkv = np.asarray(w_qkv, dtype=np.float32)
    w_proj = np.asarray(w_proj, dtype=np.float32)
    q_gain = np.asarray(q_gain, dtype=np.float32)

    cos, sin_signed, bmask, ident = _host_tables()
    nc = _get_nc()

    # token-level rms_norm commutes out of q/k (per-head renorm) and is what
    # v needs directly, so normalize once on the host
    rstd = 1.0 / np.sqrt(np.mean(x * x, axis=-1, keepdims=True) + EPS)
    x16 = (x * rstd).astype(np.float16)
    # pre-tiled transposed x: xt[b][tt, p, kt*128+j] = xhat[b][tt*128+j, kt*128+p]
    xts = []
    for b in range(B):
        x4 = x16[b].reshape(TT, P, KT, P)              # [tt, jt, kt, p]
        xts.append(np.ascontiguousarray(x4.transpose(0, 3, 2, 1)
                                        ).reshape(TT, P, C))

    in_maps = []
    for r in range(8):
        b, g = r // 4, r % 4
        wq = w_qkv[:, g * JQ:(g + 1) * JQ]
        wk = w_qkv[:, C + g * HD:C + (g + 1) * HD]
        wv = w_qkv[:, C + KV_DIM + g * HD:C + KV_DIM + (g + 1) * HD]
        wqkv_g = np.concatenate([wq, wk, wv], axis=1).astype(np.float16)
        wqkv_t = np.ascontiguousarray(
            wqkv_g.reshape(KT, P, JTOT).transpose(1, 0, 2))    # [p, kt, j]
        wproj_g = w_proj[g * JQ:(g + 1) * JQ, :].astype(np.float16)
        wproj_t = np.ascontiguousarray(
            wproj_g.reshape(REP, P, C).transpose(1, 0, 2))     # [p, h, c]
        in_maps.append({
            "xt": xts[b],
            "wqkv": wqkv_t,
            "wproj": wproj_t,
            "gain": np.ascontiguousarray(q_gain[g * REP:(g + 1) * REP].reshape(1, REP)),
            "costab": cos,
            "sintab": sin_signed,
            "bmask": bmask,
            "ident": ident,
        })

    trace = os.environ.get("KERNEL_TRACE") == "1"
    if trace:
        try:
            import antenv.axon_hooks  # noqa: F401
        except ImportError:
            trace = False
    res = run_bass_kernel_spmd(nc, in_maps, core_ids=list(range(8)), trace=trace)
    LAST_RESULT = res

    out = np.zeros((B, T, C), dtype=np.float32)
    for r in range(8):
        b = r // 4
        out[b] += res.results[r]["y"].astype(np.float32)
    return out


# revision 37
# speedup vs baseline: 1.0011x; 1.0011x over previous
"""Trainium2 Bass kernel for CausalSelfAttention (GQA + per-head RMS norm + RoPE).

Sharding: 8 cores = batch(2) x kv-head-group(4). Each core computes, for its
(b, g): qkv projection (its 4 rep q heads + 1 kv head), per-head RMS norm,
RoPE, causal attention, and a partial output projection (its 512 rows of
w_proj). Host sums the 4 partial projections per batch element.

v3 design notes (vs the v2 f16 baseline at 299us):
  - Host pre-normalizes x (token rms_norm commutes out of q/k entirely; v
    needs x-hat directly), so the 8MB/core untransposed-x input, its ACT
    square pass, and the v rescale all disappear.  Input DMA drops 23->15MB,
    fixing the DMA-saturated 21us startup stall.
  - wqkv streams as 16 per-kt slices on two queues in consumption order so
    tile-0's accumulation never waits on the bulk transfer.
  - Chunk-0 attention is woven into the tail of phase 1 (its deps are token
    tiles 0-3 only), removing the phase-boundary PE gap and the HAM
    re-throttle it caused.
  - Attention emission is split into S-steps (score MMs + exp + mask + den)
    and V-steps (attnV MMs), software-pipelined across the two heads of a
    pair so each V sits >=2.5us of PE work after its own exp.
  - Softmax denominator: partition-sum and broadcast fused into ONE matmul
    with an all-ones [128,128] stationary operand (out rows all equal the
    column sum), dropping the two-matmul ds/broadcast chain.
  - den accumulation: bulk full-width adds on GpSimd (otherwise idle in
    phase 2), ragged/diagonal adds + folds on DVE; causal masks on DVE
    (cheap 2x f16) instead of GpSimd; all y PSUM->SBUF copies on DVE so
    ACT does (almost) nothing but exp.
"""

import functools
import os

import numpy as np

from concourse import bacc, bass, mybir
from concourse import tile
from concourse.bass_utils import run_bass_kernel_spmd

# The activation-table pass binds exp to `exp_and_others` even when
# `natural_log_exp_and_others` (which also has ln + square) covers every
# function this kernel uses, causing a table reload between each ln and exp.
# Restrict exp/ln to the combined set (set order, hence set ids, preserved)
# so the whole kernel runs on one table load.
_orig_get_activation_tables = bacc.get_activation_tables


@functools.cache
def _patched_get_activation_tables(arch):
    t = dict(_orig_get_activation_tables(arch))
    keep = "natural_log_exp_and_others"
    if keep in t:
        AFT = mybir.ActivationFunctionType
        for k in t:
            if k != keep:
                t[k] = t[k] - {AFT.Exp, AFT.Ln}
    return t


bacc.get_activation_tables = _patched_get_activation_tables

# Problem shape (hardcoded per contract)
B, T, C = 2, 2048, 2048
N_HEADS, N_KV = 16, 4
HD = C // N_HEADS            # 128
REP = N_HEADS // N_KV        # 4
KV_DIM = N_KV * HD           # 512
P = 128
TT = T // P                  # 16 token tiles
KT = C // P                  # 16 contraction tiles
JQ = REP * HD                # 512 local q cols
JTOT = JQ + 2 * HD           # 768 local qkv cols
TCW = 512                    # attention t-chunk width
NTC = T // TCW               # 4
EPS = 1.1920929e-07
SCALE = 1.0 / float(np.sqrt(HD))
EXPBIAS = -9.0               # et = exp(s*SCALE - 9) stays in f16 range

F32 = mybir.dt.float32
F16 = mybir.dt.float16
AF = mybir.ActivationFunctionType
AX = mybir.AxisListType


def _emit(nc):
    # xt[tt, p, kt*128 + j] = xhat[tt*128 + j, kt*128 + p]  (pre-tiled lhsT,
    # host-normalized: xhat = x * rstd(token))
    xt_d = nc.dram_tensor("xt", [TT, P, C], F16, kind="ExternalInput")
    # wqkv[p, kt, j] = w_qkv[kt*128 + p, j]; j = [q 512 | k 128 | v 128]
    wqkv_d = nc.dram_tensor("wqkv", [P, KT, JTOT], F16, kind="ExternalInput")
    # wproj[p, h, c] = w_proj[h*128 + p, c]
    wproj_d = nc.dram_tensor("wproj", [P, REP, C], F16, kind="ExternalInput")
    gain_d = nc.dram_tensor("gain", [1, REP], F32, kind="ExternalInput")
    # rope tables pre-expanded to all 5 lanes (4 q heads + k):
    # cos5[tt, p, j*HD+d] = cos(tt*128+p, d)
    cos_d = nc.dram_tensor("costab", [TT, P, 5 * HD], F16, kind="ExternalInput")
    sin_d = nc.dram_tensor("sintab", [TT, P, 5 * HD], F16, kind="ExternalInput")  # [:, :, :64] = -sin
    bmask_d = nc.dram_tensor("bmask", [P, P], F16, kind="ExternalInput")  # 0/1
    id_d = nc.dram_tensor("ident", [P, P], F16, kind="ExternalInput")
    y_d = nc.dram_tensor("y", [T, C], F16, kind="ExternalOutput")

    with tile.TileContext(nc) as tc:
        with tc.tile_pool(name="persist", bufs=1) as pp, \
             tc.tile_pool(name="psum", bufs=1, space="PSUM") as psp, \
             nc.allow_low_precision(reason="f16 kernel by design"):
            # Long-lived f16 activations
            qTall = pp.tile([P, REP, T], F16, name="qTall", tag="qTall")
            kTt = pp.tile([P, T], F16, name="kTt", tag="kTt")
            vN = pp.tile([P, TT, HD], F16, name="vN", tag="vN")
            bmask_sb = pp.tile([P, P], F16, name="bmask_sb", tag="bmask")
            nc.gpsimd.dma_start(out=bmask_sb, in_=bmask_d.ap())
            negb = pp.tile([P, 1], F32, name="negb", tag="negb")
            nc.vector.memset(negb, EXPBIAS)
            ones128 = pp.tile([P, P], F16, name="ones128", tag="ones128")
            nc.vector.memset(ones128, 1.0)
            wproj_sb = pp.tile([P, REP, C], F16, name="wproj_sb", tag="wproj")

            # ---------------- Phase 1: qkv + norms + rope + transposes -------
            with tc.tile_pool(name="ph1", bufs=1) as p1:
                wqkv_sb = p1.tile([P, KT, JTOT], F16, name="wqkv_sb", tag="wqkv")
                id_sb = p1.tile([P, P], F16, name="id_sb", tag="ident")
                nc.gpsimd.dma_start(out=id_sb, in_=id_d.ap())
                eps_t = p1.tile([P, 1], F32, name="eps_t", tag="eps")
                nc.vector.memset(eps_t, EPS)

                # HAM warm-up: the first ~13us are DMA/preamble-bound with
                # PE idle, so the clock gate would hold the PE at 1.2GHz for
                # the first ~3.4us of real work.  Chew cheap matmuls on the
                # memset ones tile (no DMA dependency -- starts the moment
                # the framework preamble ends) to flip HAM to 8/8 and keep
                # it there until tile 0's inputs land.
                warm_z = p1.tile([P, P], F16, name="warm_z", tag="warm_z")
                nc.vector.memset(warm_z, 0.0)
                warm_sb = p1.tile([P, 1], F32, name="warm_sb", tag="warm_sb")
                for g in range(4):
                    warm_ps = psp.tile([P, P], F32, name=f"warm_ps{g}",
                                       tag="small", bufs=2)
                    for i in range(22):
                        nc.tensor.matmul(warm_ps, warm_z, warm_z,
                                         start=(i == 0), stop=(i == 21))
                    nc.vector.tensor_copy(warm_sb, warm_ps[:, 0:1])
                # preload the ln/exp table set (the only set this kernel
                # uses) while ACT is otherwise idle
                nc.scalar.activation(warm_sb, eps_t, AF.Ln)

                # broadcast gain [1,4] -> [128,4] via 0-stride DMA
                # replication (DMA emitted after the critical startup DMAs)
                gainb = p1.tile([P, REP], F32, name="gainb", tag="gainb")


                # chunk-0 attention woven into the phase-1 tail: S-steps and
                # V-steps land on different tiles so each exp has a full
                # tile (~6us) of latency cover; attnV drains per-block to an
                # SBUF accumulator so no long-lived PSUM tile ever blocks
                # the per-tile q_ps/kv_ps/tq rotations.
                def chunk0_steps():
                    for h in range(REP):
                        osb = pp.tile([P, TCW], F32, name=f"c0osb_{h}",
                                      tag="c0osb", bufs=2)
                        sA, vA, tail = attend_plan(0, h, None, c0_osb=osb)
                        yield sA[0]
                        yield sA[1]
                        yield vA[0]
                        yield vA[1]
                        yield tail

                c0_iter = None

                # Token tiles in an order that retires the late-chunk tiles
                # (12-15) early: the phase boundary then depends only on
                # tiles that finished long ago, and chunk 0/1 deps (tiles
                # 0-7) are ready the moment phase 2 starts.
                proc_order = [0, 1, 2, 3, 12, 13, 14, 15,
                              4, 5, 6, 7, 8, 9, 10, 11]

                # Startup DMA priority: tile-0's xt on the otherwise-idle
                # sync queue; ALL wqkv kt-slices next (tile 0 consumes them
                # in order); the next two xt tiles land behind wqkv on the
                # same queues so they cannot steal its bandwidth.
                xT_tiles = {}

                def xT_dma(oi, qeng):
                    tt = proc_order[oi]
                    xt = p1.tile([P, C], F16, name=f"xT_{tt}", tag="xT", bufs=3)
                    qeng.dma_start(out=xt, in_=xt_d.ap()[tt])
                    xT_tiles[oi] = xt

                xT_dma(0, nc.sync)
                for ks in range(KT // 2):
                    qeng = (nc.scalar, nc.gpsimd)[ks % 2]
                    qeng.dma_start(out=wqkv_sb[:, 2 * ks:2 * ks + 2, :],
                                   in_=wqkv_d.ap()[:, 2 * ks:2 * ks + 2, :])
                xT_dma(1, nc.scalar)
                xT_dma(2, nc.gpsimd)
                nc.scalar.dma_start(out=gainb,
                                    in_=gain_d.ap()[0].partition_broadcast(P))

                tpq = []  # software-pipelined q/k transposes (depth 2)
                H2 = HD // 2

                def emit_qk_transposes(qkf_t, ptt, last=False):
                    tq = psp.tile([P, JQ], F16, name=f"tq_{ptt}", tag="mm", bufs=2)
                    for h in range(REP):
                        nc.tensor.transpose(tq[:, h * P:(h + 1) * P],
                                            qkf_t[:, h * P:(h + 1) * P], id_sb)
                    # ONE strided PSUM->SBUF copy for all 4 heads; on ACT
                    # ('copy' is in every table set) except at the phase
                    # boundary where ACT is the bottleneck queue
                    ceng = nc.vector.tensor_copy if last else nc.scalar.copy
                    ceng(qTall[:, :, ptt * P:(ptt + 1) * P],
                         tq.rearrange("p (h c) -> p h c", h=REP))
                    tk = psp.tile([P, HD], F16, name=f"tk_{ptt}", tag="small", bufs=2)
                    nc.tensor.transpose(tk, qkf_t[:, JQ:JQ + HD], id_sb)
                    ceng(kTt[:, ptt * P:(ptt + 1) * P], tk)

                ao_tiles = {}

                def attend_plan(tci, h, o_ps, c0_osb=None, o_blocks=None):
                    """Return (s_steps, v_steps, tail) closures for (tci, h).

                    s_steps[k]: score MMs + exp (+ causal mask) + den update
                    v_steps[k]: the two attnV MMs consuming et[k]
                    tail: den fold + fused partition-sum/broadcast + rescale

                    The last 4 s-tiles form the diagonal block: their score /
                    attnV matmuls are column-sliced to the causal range
                    (widths 512/384/256/128) and only the leading [128,128]
                    triangle of each strip needs masking.

                    c0_osb: chunk-0 weave mode -- attnV uses transient psum
                    blocks drained into this SBUF f32 accumulator so the
                    phase-1 psum rotations never block on a long-lived
                    accumulator."""
                    nst = 4 * (tci + 1)
                    nfull = nst - 4
                    denf = pp.tile([P, 2 * TCW], F16, name=f"dnf_{tci}_{h}",
                                   tag="denf", bufs=4)
                    qTc = qTall[:, h, tci * TCW:(tci + 1) * TCW]
                    s_steps, v_steps = [], []
                    for sw in range(nfull // 2):
                        st0, st1 = 2 * sw, 2 * sw + 1
                        sc = psp.tile([P, 2 * TCW], F32, name=f"sc_{tci}_{h}_{sw}",
                                      tag="mm", bufs=2)
                        et = pp.tile([P, 2 * TCW], F16, name=f"et_{tci}_{h}_{sw}",
                                     tag="et", bufs=6)

                        def s_fn(sc=sc, et=et, st0=st0, st1=st1, sw=sw):
                            nc.tensor.matmul(sc[:, 0:TCW],
                                             kTt[:, st0 * P:(st0 + 1) * P],
                                             qTc, start=True, stop=True)
                            nc.tensor.matmul(sc[:, TCW:],
                                             kTt[:, st1 * P:(st1 + 1) * P],
                                             qTc, start=True, stop=True)
                            nc.scalar.activation(et, sc, AF.Exp, scale=SCALE,
                                                 bias=negb)
                            if sw == 0:
                                nc.vector.tensor_copy(denf, et)
                            else:
                                nc.vector.tensor_add(denf, denf, et)

                        def v_fn(et=et, st0=st0, st1=st1, sw=sw):
                            nc.tensor.matmul(o_ps, vN[:, st0, :], et[:, 0:TCW],
                                             start=(sw == 0), stop=False)
                            nc.tensor.matmul(o_ps, vN[:, st1, :], et[:, TCW:],
                                             start=False, stop=False)

                        s_steps.append(s_fn)
                        v_steps.append(v_fn)
                    first = (nfull == 0)
                    for pi, (v0, v1) in enumerate(((0, 1), (2, 3))):
                        st0, st1 = 4 * tci + v0, 4 * tci + v1
                        t0, t1 = v0 * P, v1 * P
                        w0, w1 = TCW - t0, TCW - t1
                        sc = psp.tile([P, 2 * TCW], F32, name=f"scd_{tci}_{h}_{pi}",
                                      tag="mm", bufs=2)
                        et = pp.tile([P, 2 * TCW], F16, name=f"etd_{tci}_{h}_{pi}",
                                     tag="et", bufs=6)

                        def s_fn(sc=sc, et=et, st0=st0, st1=st1, pi=pi,
                                 t0=t0, t1=t1, w0=w0, w1=w1):
                            nc.tensor.matmul(sc[:, 0:w0],
                                             kTt[:, st0 * P:(st0 + 1) * P],
                                             qTc[:, t0:TCW], start=True, stop=True)
                            nc.tensor.matmul(sc[:, w0:w0 + w1],
                                             kTt[:, st1 * P:(st1 + 1) * P],
                                             qTc[:, t1:TCW], start=True, stop=True)
                            nc.scalar.activation(et[:, 0:w0 + w1], sc[:, 0:w0 + w1],
                                                 AF.Exp, scale=SCALE, bias=negb)
                            # ragged triangle masks: first 128 cols of each
                            # strip, on GpSimd (otherwise idle in phase 2)
                            nc.gpsimd.tensor_mul(et[:, 0:P], et[:, 0:P], bmask_sb)
                            nc.gpsimd.tensor_mul(et[:, w0:w0 + P],
                                                 et[:, w0:w0 + P], bmask_sb)
                            if first and pi == 0:
                                nc.vector.tensor_copy(denf[:, 0:TCW], et[:, 0:TCW])
                                nc.vector.memset(denf[:, TCW:TCW + t1], 0.0)
                                nc.vector.tensor_copy(denf[:, TCW + t1:2 * TCW],
                                                      et[:, w0:w0 + w1])
                            else:
                                nc.vector.tensor_add(denf[:, t0:TCW],
                                                     denf[:, t0:TCW], et[:, 0:w0])
                                nc.vector.tensor_add(denf[:, TCW + t1:2 * TCW],
                                                     denf[:, TCW + t1:2 * TCW],
                                                     et[:, w0:w0 + w1])

                        if c0_osb is None:
                            def v_fn(et=et, st0=st0, st1=st1, pi=pi,
                                     t0=t0, t1=t1, w0=w0, w1=w1):
                                nc.tensor.matmul(o_ps[:, t0:TCW], vN[:, st0, :],
                                                 et[:, 0:w0],
                                                 start=(first and pi == 0),
                                                 stop=False,
                                                 skip_group_check=True)
                                nc.tensor.matmul(o_ps[:, t1:TCW], vN[:, st1, :],
                                                 et[:, w0:w0 + w1],
                                                 start=False, stop=(pi == 1),
                                                 skip_group_check=True)
                        else:
                            def v_fn(et=et, st0=st0, st1=st1, pi=pi,
                                     t0=t0, t1=t1, w0=w0, w1=w1):
                                o_blk = psp.tile([P, TCW], F32,
                                                 name=f"c0o_{h}_{pi}",
                                                 tag="acc", bufs=2)
                                nc.tensor.matmul(o_blk[:, t0:TCW], vN[:, st0, :],
                                                 et[:, 0:w0],
                                                 start=True, stop=False,
                                                 skip_group_check=True)
                                nc.tensor.matmul(o_blk[:, t1:TCW], vN[:, st1, :],
                                                 et[:, w0:w0 + w1],
                                                 start=False, stop=True,
                                                 skip_group_check=True)
                                if pi == 0:
                                    nc.vector.tensor_copy(c0_osb, o_blk)
                                else:
                                    nc.vector.tensor_add(c0_osb[:, t0:TCW],
                                                         c0_osb[:, t0:TCW],
                                                         o_blk[:, t0:TCW])

                        s_steps.append(s_fn)
                        v_steps.append(v_fn)

                    def tail():
                        den_r = pp.tile([P, TCW], F16, name=f"dnr_{tci}_{h}",
                                        tag="denr", bufs=4)
                        nc.vector.tensor_add(den_r, denf[:, 0:TCW], denf[:, TCW:])
                        # all-ones stationary: every out row = column sum(den_r)
                        rb_ps = psp.tile([P, TCW], F32, name=f"rb_{tci}_{h}",
                                         tag="small", bufs=2)
                        nc.tensor.matmul(rb_ps, ones128, den_r, start=True,
                                         stop=True)
                        rb = pp.tile([P, TCW], F32, name=f"rbs_{tci}_{h}",
                                     tag="rb", bufs=4)
                        nc.vector.reciprocal_approx_fast(rb, rb_ps)
                        aot = pp.tile([P, TCW], F16, name=f"ao_{tci}_{h}",
                                      tag="ao", bufs=8)
                        osrc = o_ps if c0_osb is None else c0_osb
                        nc.vector.tensor_mul(aot, osrc, rb)
                        ao_tiles[(tci, h)] = aot

                    return s_steps, v_steps, tail

                def pair_group_steps(tci, hp):
                    """Emission schedule for heads (hp, hp+1) of chunk tci.

                    S/V stages are split and cross-staggered so every V sits
                    ~2.5us of PE work after its own S (exp+mask latency)."""
                    o0 = psp.tile([P, TCW], F32, name=f"ops_{tci}_{hp}",
                                  tag="acc", bufs=2)
                    o1 = psp.tile([P, TCW], F32, name=f"ops_{tci}_{hp + 1}",
                                  tag="acc", bufs=2)
                    sA, vA, tA = attend_plan(tci, hp, o0)
                    sB, vB, tB = attend_plan(tci, hp + 1, o1)
                    n = len(sA)
                    sA[0]()
                    sB[0]()
                    yield
                    for k in range(n):
                        vA[k]()
                        if k + 1 < n:
                            sA[k + 1]()
                        yield
                        vB[k]()
                        if k + 1 < n:
                            sB[k + 1]()
                        yield
                    tA()
                    yield
                    tB()
                    yield

                def proj_steps(tci):
                    """Projection for chunk tci as small emit-steps (4 MMs each)."""
                    for ttl in range(4):
                        yt = pp.tile([P, C], F16, name=f"y_{tci}_{ttl}", tag="y", bufs=2)
                        for ncs in range(4):
                            def step(tci=tci, ttl=ttl, ncs=ncs, yt=yt):
                                y_ps = psp.tile([P, 512], F32,
                                                name=f"yps_{tci}_{ttl}_{ncs}",
                                                tag="small", bufs=2)
                                for h in range(REP):
                                    nc.tensor.matmul(
                                        y_ps,
                                        ao_tiles[(tci, h)][:, ttl * P:(ttl + 1) * P],
                                        wproj_sb[:, h, ncs * 512:(ncs + 1) * 512],
                                        start=(h == 0), stop=(h == REP - 1))
                                # alternate DVE/ACT for the PSUM->SBUF y copies
                                if ncs % 2 == 0:
                                    nc.vector.tensor_copy(
                                        yt[:, ncs * 512:(ncs + 1) * 512], y_ps)
                                else:
                                    nc.scalar.copy(
                                        yt[:, ncs * 512:(ncs + 1) * 512], y_ps)
                                if tci == NTC - 1:
                                    # last chunk: DMA per column slice so the
                                    # trailing drain after the final matmul
                                    # is short
                                    row = (tci * 4 + ttl) * P
                                    qeng = (nc.sync, nc.gpsimd, nc.sync,
                                            nc.gpsimd)[ncs]
                                    qeng.dma_start(
                                        out=y_d.ap()[row:row + P,
                                                     ncs * 512:(ncs + 1) * 512],
                                        in_=yt[:, ncs * 512:(ncs + 1) * 512])
                            yield step
                        if tci != NTC - 1:
                            def dma_step(tci=tci, ttl=ttl, yt=yt):
                                row = (tci * 4 + ttl) * P
                                qeng = (nc.sync, nc.gpsimd, nc.sync, nc.sync)[ttl]
                                qeng.dma_start(out=y_d.ap()[row:row + P, :], in_=yt)
                            yield dma_step

                NQK = REP + 1     # merged q-heads + k lane count

                for oi in range(TT):
                    tt = proc_order[oi]
                    if oi + 3 < TT:
                        # bufs=3 rotation throttles this DMA until tile oi's
                        # buffer frees, so prefetches never starve wqkv
                        xT_dma(oi + 3, nc.sync)
                    xT_t = xT_tiles.pop(oi)
                    cos_t = p1.tile([P, NQK * HD], F16, name=f"cos_{tt}",
                                    tag="cos", bufs=2)
                    sin_t = p1.tile([P, NQK * HD], F16, name=f"sin_{tt}",
                                    tag="sin", bufs=2)
                    ceng = nc.gpsimd if oi < 2 else nc.sync
                    ceng.dma_start(out=cos_t, in_=cos_d.ap()[tt])
                    ceng.dma_start(out=sin_t, in_=sin_d.ap()[tt])
                    if 8 <= oi <= 11:  # proj weights, one head per tile so a
                        # single large transfer never backs up the DMA engines
                        h = oi - 8
                        qeng = (nc.scalar, nc.gpsimd)[h % 2]
                        qeng.dma_start(out=wproj_sb[:, h:h + 1, :],
                                       in_=wproj_d.ap()[:, h:h + 1, :])

                    # qkv matmuls: q_ps [P, 512], kv_ps [P, 256]
                    q_ps = psp.tile([P, JQ], F32, name=f"qps_{tt}", tag="acc", bufs=2)
                    kv_ps = psp.tile([P, 2 * HD], F32, name=f"kvps_{tt}", tag="small", bufs=2)
                    for kt in range(KT):
                        lb = xT_t[:, kt * P:(kt + 1) * P]
                        nc.tensor.matmul(q_ps, lb, wqkv_sb[:, kt, 0:JQ],
                                         start=(kt == 0), stop=(kt == KT - 1))
                        nc.tensor.matmul(kv_ps, lb, wqkv_sb[:, kt, JQ:JTOT],
                                         start=(kt == 0), stop=(kt == KT - 1))

                    if oi < 3:
                        # zero-power filler matmuls: bridge the DMA-paced
                        # early tiles so the clock gate never re-throttles
                        wps = psp.tile([P, P], F32, name=f"wf_{oi}",
                                       tag="small", bufs=2)
                        for i in range(12):
                            nc.tensor.matmul(wps, warm_z, warm_z,
                                             start=(i == 0), stop=(i == 11))
                        nc.vector.tensor_copy(warm_sb, wps[:, 0:1])

                    # 2-tiles-back q/k transposes overlap this tile's matmuls
                    if len(tpq) >= 2:
                        emit_qk_transposes(*tpq.pop(0))

                    # ---- q: per-head rms norm (x gain) ----
                    qsq = p1.tile([P, JQ], F16, name=f"qsq_{tt}", tag="qsq", bufs=2)
                    nc.scalar.activation(qsq, q_ps, AF.Square)
                    ssq4 = p1.tile([P, REP], F32, name=f"ssq4_{tt}", tag="ssq4", bufs=2)
                    nc.vector.reduce_sum(ssq4, qsq.rearrange("p (h d) -> p h d", h=REP),
                                         axis=AX.X)
                    # rstd = exp(-0.5*ln(ms+eps)): keeps ACT on the single
                    # ln/exp table set (Sqrt would force a table reload)
                    lq = p1.tile([P, REP], F32, name=f"lq_{tt}", tag="lq", bufs=2)
                    nc.scalar.activation(lq, ssq4, AF.Ln, scale=1.0 / HD, bias=eps_t)
                    rstdq = p1.tile([P, REP], F32, name=f"rstdq_{tt}", tag="rstdq", bufs=2)
                    nc.scalar.activation(rstdq, lq, AF.Exp, scale=-0.5)
                    rstdqg = p1.tile([P, REP], F32, name=f"rstdqg_{tt}", tag="rstdqg", bufs=2)
                    nc.vector.tensor_mul(rstdqg, rstdq, gainb)

                    # ---- k: rms norm (ACT sums) ----
                    scrk = p1.tile([P, HD], F16, name=f"scrk_{tt}", tag="scrk", bufs=2)
                    ssk = p1.tile([P, 1], F32, name=f"ssk_{tt}", tag="ssk", bufs=2)
                    nc.scalar.activation(scrk, kv_ps[:, 0:HD], AF.Square,
                                         accum_out=ssk)
                    lk = p1.tile([P, 1], F32, name=f"lk_{tt}", tag="lk", bufs=2)
                    nc.scalar.activation(lk, ssk, AF.Ln, scale=1.0 / HD, bias=eps_t)
                    rstdk = p1.tile([P, 1], F32, name=f"rstdk_{tt}", tag="rstdk", bufs=2)
                    nc.scalar.activation(rstdk, lk, AF.Exp, scale=-0.5)

                    # ---- merged q+k normalize into one [P, 5*HD] tile, then
                    # one rope pass over all 5 lanes (DVE f-mul, GpSimd
                    # B-muls + add) ----
                    qkn_t = p1.tile([P, NQK * HD], F16, name=f"qkn_{tt}", tag="qkn", bufs=2)
                    # all 4 q heads scaled in one op (rstdqg broadcast along d)
                    nc.vector.tensor_mul(
                        qkn_t[:, 0:JQ].rearrange("p (h d) -> p h d", h=REP),
                        q_ps.rearrange("p (h d) -> p h d", h=REP),
                        rstdqg[:, :, None].broadcast_to([P, REP, HD]))
                    nc.vector.tensor_scalar_mul(qkn_t[:, JQ:JQ + HD],
                                                kv_ps[:, 0:HD], rstdk)
                    qkn3 = qkn_t.rearrange("p (h d) -> p h d", h=NQK)
                    qkf_t = p1.tile([P, NQK * HD], F16, name=f"qkf_{tt}", tag="qkf", bufs=3)
                    qkf3 = qkf_t.rearrange("p (h d) -> p h d", h=NQK)
                    qkB_t = p1.tile([P, NQK * HD], F16, name=f"qkB_{tt}", tag="qkB", bufs=2)
                    qkB3 = qkB_t.rearrange("p (h d) -> p h d", h=NQK)
                    sin3 = sin_t.rearrange("p (h d) -> p h d", h=NQK)
                    # full rope on DVE at 2x f16 rate (expanded tables keep
                    # every operand contiguous)
                    nc.vector.tensor_mul(qkf_t, qkn_t, cos_t)
                    nc.vector.tensor_mul(qkB3[:, :, 0:H2], qkn3[:, :, H2:HD],
                                         sin3[:, :, 0:H2])
                    nc.vector.tensor_mul(qkB3[:, :, H2:HD], qkn3[:, :, 0:H2],
                                         sin3[:, :, H2:HD])
                    nc.vector.tensor_add(qkf_t, qkf_t, qkB_t)

                    # ---- v: already token-normalized on host; just cast ----
                    nc.vector.tensor_copy(vN[:, tt, :], kv_ps[:, HD:2 * HD])

                    tpq.append((qkf_t, tt))

                    if oi == 5:
                        c0_iter = chunk0_steps()
                    if c0_iter is not None:
                        for _ in range(2):
                            st = next(c0_iter, None)
                            if st is not None:
                                st()

                # the last two tiles' transposes are only needed by chunk 2;
                # weave them into early phase 2 so the first score matmuls
                # don't rotate PSUM slots behind them
                deferred_tq = [
                    (lambda a=args: emit_qk_transposes(*a, last=True))
                    for args in tpq]
                tpq.clear()
                if c0_iter is not None:
                    for st in c0_iter:
                        st()

            # ---------------- Phase 2+3: attention + projection --------------
            if True:

                class Weaver:
                    """Spread a step list evenly over a known yield count so
                    late pair-tails still have PE filler work."""

                    def __init__(self, steps, yields):
                        self.steps = list(steps)
                        self.y = max(yields, 1)
                        self.i = 0
                        self.t = 0

                    def tick(self):
                        self.t += 1
                        want = min(len(self.steps),
                                   (len(self.steps) * self.t) // self.y)
                        while self.i < want:
                            self.steps[self.i]()
                            self.i += 1

                    def flush(self):
                        while self.i < len(self.steps):
                            self.steps[self.i]()
                            self.i += 1

                def chunk_yields(tci):
                    return 8 * tci + 14

                wv = Weaver(deferred_tq + list(proj_steps(0)),
                            chunk_yields(1))
                for tci in range(1, NTC):
                    for hp in (0, 2):
                        for _ in pair_group_steps(tci, hp):
                            wv.tick()
                    wv.flush()
                    wv = Weaver(proj_steps(tci),
                                chunk_yields(tci + 1) if tci + 1 < NTC else 1)
                wv.flush()

    return nc


_NC_CACHE = {}
LAST_RESULT = None


def _get_nc():
    if "v3" not in _NC_CACHE:
        nc = bacc.Bacc("TRN2", target_bir_lowering=False, debug=False)
        _emit(nc)
        nc.compile()
        _NC_CACHE["v3"] = nc
    return _NC_CACHE["v3"]


def _host_tables():
    inv_freq = 1.0 / (10000.0 ** (np.arange(0, HD, 2, dtype=np.float64) / HD))
    t = np.arange(T, dtype=np.float64)
    freqs = np.outer(t, inv_freq)                      # [T, 64]
    emb = np.concatenate([freqs, freqs], axis=-1)      # [T, 128]
    cos = np.cos(emb).astype(np.float16)
    sin = np.sin(emb).astype(np.float32)
    sin_signed = sin.copy()
    sin_signed[:, :HD // 2] *= -1.0                    # first half gets -sin
    sin_signed = sin_signed.astype(np.float16)
    # expand to the 5 merged rope lanes (4 q heads + k), tiled by token tile
    cos5 = np.tile(cos, (1, 5)).reshape(TT, P, 5 * HD)
    sin5 = np.tile(sin_signed, (1, 5)).reshape(TT, P, 5 * HD)
    # binary causal triangle for the leading 128 cols of each diagonal strip
    s = np.arange(P)[:, None]
    tcol = np.arange(P)[None, :]
    bmask = (s <= tcol).astype(np.float16)
    ident = np.eye(P, dtype=np.float16)
    return np.ascontiguousarray(cos5), np.ascontiguousarray(sin5), bmask, ident


def kernel(x, w_qkv, w_proj, q_gain):
    global LAST_RESULT
    x = np.asarray(x, dtype=np.float32)
    w_qkv = np.asarray(w_qkv, dtype=np.float32)
    w_proj = np.asarray(w_proj, dtype=np.float32)
    q_gain = np.asarray(q_gain, dtype=np.float32)

    cos, sin_signed, bmask, ident = _host_tables()
    nc = _get_nc()

    # token-level rms_norm commutes out of q/k (per-head renorm) and is what
    # v needs directly, so normalize once on the host
    rstd = 1.0 / np.sqrt(np.mean(x * x, axis=-1, keepdims=True) + EPS)
    x16 = (x * rstd).astype(np.float16)
    # pre-tiled transposed x: xt[b][tt, p, kt*128+j] = xhat[b][tt*128+j, kt*128+p]
    xts = []
    for b in range(B):
        x4 = x16[b].reshape(TT, P, KT, P)              # [tt, jt, kt, p]
        xts.append(np.ascontiguousarray(x4.transpose(0, 3, 2, 1)
                                        ).reshape(TT, P, C))

    in_maps = []
    for r in range(8):
        b, g = r // 4, r % 4
        wq = w_qkv[:, g * JQ:(g + 1) * JQ]
        wk = w_qkv[:, C + g * HD:C + (g + 1) * HD]
        wv = w_qkv[:, C + KV_DIM + g * HD:C + KV_DIM + (g + 1) * HD]
        wqkv_g = np.concatenate([wq, wk, wv], axis=1).astype(np.float16)
        wqkv_t = np.ascontiguousarray(
            wqkv_g.reshape(KT, P, JTOT).transpose(1, 0, 2))    # [p, kt, j]
        wproj_g = w_proj[g * JQ:(g + 1) * JQ, :].astype(np.float16)
        wproj_t = np.ascontiguousarray(
            wproj_g.reshape(REP, P, C).transpose(1, 0, 2))     # [p, h, c]
        in_maps.append({
            "xt": xts[b],
            "wqkv": wqkv_t,
            "wproj": wproj_t,
            "gain": np.ascontiguousarray(q_gain[g * REP:(g + 1) * REP].reshape(1, REP)),
            "costab": cos,
            "sintab": sin_signed,
            "bmask": bmask,
            "ident": ident,
        })

    trace = os.environ.get("KERNEL_TRACE") == "1"
    if trace:
        try:
            import antenv.axon_hooks  # noqa: F401
        except ImportError:
            trace = False
    res = run_bass_kernel_spmd(nc, in_maps, core_ids=list(range(8)), trace=trace)
    LAST_RESULT = res

    out = np.zeros((B, T, C), dtype=np.float32)
    for r in range(8):
        b = r // 4
        out[b] += res.results[r]["y"].astype(np.float32)
    return out
